# revision 1
# baseline (speedup 1.0000x reference)
"""Trainium2 Bass kernel: 2-layer GRU (H=128) over 28 timesteps + Linear head.

Reference computation (PyTorch GRUCell semantics, gates r,z,n):
    for t in 28 rows of each 28x28 image:
        h1 = relu(gru1(x_t, h1));  h2 = relu(gru2(h1, h2))
    out = h2 @ w_out.T + b_out

Sharding: pure data parallel, batch 32768 -> 8 cores x 4096.
On-chip layout: transposed [hidden=partition, batch=free]; batch tiled 8x512.

Software-pipelined schedule (cell = one GRU cell step for one batch
subtile; same-layer cell pairs of L1(t) and L2(t-1) interleaved so every
producer->consumer distance is 10-16 slots -- the elementwise tail has ~10
slots of margin before the consuming matmuls, which keeps the PE free of
rhs stalls even when the device thermally throttles). Adjacent cell pairs (2m, 2m+1) share FD=1024 SBUF pair
tiles so tanh/relu/t3/t4/hp run as one instruction per pair (amortizes the
~180ns per-instruction overhead on Act/DVE); the sigmoid writes a segmented
[r(a)|r(b)|z(a)|z(b)] pair layout so the z-side multiply pairs too.
Per slot k:
    PE:   6 matmuls of cell k            (rz x/h accum pairs, gi, gh)
    Act:  tanh-pair(k-4), sigmoid(k), relu-pair(k-7)
          (sig before relu: sig gates DVE's t1; relu is 7 slots stale)
    DVE:  t3/t4/hp-pair(k-5), t1(k-1), t2(k-1)
PSUM: rz bufs=2 (4 banks) + gi bufs=2 + gh bufs=2 = 8 banks exactly.
GpSimd (Q7) is deliberately unused: its elementwise ops run 7-10x slower
than modeled AND its SBUF traffic degrades concurrent DVE ops ~10x.

Bias folding:
  - L1: x augmented with ones row; w1aug row 28 carries b_ih1(+b_hh1 for r,z)
    and b_ih1n; L1 sigmoid is one [128,1024] ACT with no bias, tanh bias 0.
  - L2: r/z biases via ScalarE activation bias (2 ACTs); b_ih2n via tanh bias.
  - b_hh*n folded into the (ghn + b) * r fused scalar_tensor_tensor.
"""

import json
import os
from contextlib import ExitStack

import ml_dtypes
import numpy as np

import concourse.bass as bass
import concourse.tile as tile
from concourse import mybir
from concourse.bass_utils import run_bass_kernel_spmd

HID = 128
T = 28
C = 28
KAUG = C + 1
NCORES = 8
N_TOTAL = 32768
B_CORE = N_TOTAL // NCORES  # 4096
BF = 512                    # batch tile (matmul free dim / psum bank)
NSUB = B_CORE // BF         # 8
NOUT = 10

F32 = mybir.dt.float32
BF16 = mybir.dt.bfloat16
AF = mybir.ActivationFunctionType
ALU = mybir.AluOpType

# stash of the last run's perf results for test harness inspection
LAST_RESULT = None


def _split_multi_waits(bir_bytes: bytes) -> bytes:
    """This walrus build rejects instructions carrying >1 sync wait
    ("Too many sync wait commands"). Split extras into standalone
    single-wait EventSemaphore instructions on the same engine, placed
    immediately before -- semantically identical blocking."""
    d = json.loads(bir_bytes)
    ctr = 0
    for fn in d["functions"]:
        for bb in fn["blocks"]:
            out = []
            for inst in bb["instructions"]:
                si = inst.get("sync_info")
                waits = (si or {}).get("on_wait") or []
                if len(waits) > 1:
                    for w in waits[:-1]:
                        ctr += 1
                        out.append({
                            "debug": inst.get("debug", 0),
                            "engine": inst.get("engine"),
                            "ins": [],
                            "outs": [],
                            "name": f"xw-{ctr}",
                            "opcode": "EventSemaphore",
                            "sync_info": {"on_update": [], "on_wait": [w]},
                        })
                    si["on_wait"] = [waits[-1]]
                out.append(inst)
            bb["instructions"] = out
    return json.dumps(d).encode()


def _build_bass() -> bass.Bass:
    nc = bass.Bass()

    x = nc.dram_tensor("x", [(T + 2) // 3, 128, B_CORE], BF16, kind="ExternalInput")
    w1aug_d = nc.dram_tensor("w1aug", [128, 3 * HID], BF16, kind="ExternalInput")
    whh1_d = nc.dram_tensor("whh1T", [HID, 3 * HID], BF16, kind="ExternalInput")
    wih2_d = nc.dram_tensor("wih2T", [HID, 3 * HID], BF16, kind="ExternalInput")
    whh2_d = nc.dram_tensor("whh2T", [HID, 3 * HID], BF16, kind="ExternalInput")
    wout_d = nc.dram_tensor("woutT", [HID, NOUT], BF16, kind="ExternalInput")
    # bias columns: 0=b2r, 1=b2z, 2=b_hh1n, 3=b_hh2n, 4=b_ih2n
    bias_d = nc.dram_tensor("biases", [HID, 5], F32, kind="ExternalInput")
    bout_d = nc.dram_tensor("bout", [NOUT, 2 * BF], F32, kind="ExternalInput")
    out_d = nc.dram_tensor("out", [NOUT, B_CORE], F32, kind="ExternalOutput")

    with ExitStack() as ctx:
        tc = ctx.enter_context(tile.TileContext(nc))

        consts = ctx.enter_context(tc.tile_pool(name="consts", bufs=1))
        prz = ctx.enter_context(tc.tile_pool(name="prz", bufs=2, space="PSUM"))
        pgi = ctx.enter_context(tc.tile_pool(name="pgi", bufs=2, space="PSUM"))
        pgh = ctx.enter_context(tc.tile_pool(name="pgh", bufs=2, space="PSUM"))
        spool = ctx.enter_context(tc.tile_pool(name="sp", bufs=2))
        hpool = ctx.enter_context(tc.tile_pool(name="hp", bufs=3))
        opool = ctx.enter_context(tc.tile_pool(name="op", bufs=1))

        w1 = consts.tile([128, 3 * HID], BF16)
        nc.sync.dma_start(out=w1, in_=w1aug_d[:, :])
        wh1 = consts.tile([HID, 3 * HID], BF16)
        nc.sync.dma_start(out=wh1, in_=whh1_d[:, :])
        wi2 = consts.tile([HID, 3 * HID], BF16)
        nc.sync.dma_start(out=wi2, in_=wih2_d[:, :])
        wh2 = consts.tile([HID, 3 * HID], BF16)
        nc.sync.dma_start(out=wh2, in_=whh2_d[:, :])
        wo = consts.tile([HID, NOUT], BF16)
        nc.sync.dma_start(out=wo, in_=wout_d[:, :])
        bs = consts.tile([HID, 5], F32)
        nc.sync.dma_start(out=bs, in_=bias_d[:, :])
        bo = consts.tile([NOUT, 2 * BF], F32)
        nc.sync.dma_start(out=bo, in_=bout_d[:, :])

        xg = []
        for g in range((T + 2) // 3):
            xt_ = consts.tile([128, B_CORE], BF16, tag=f"xg_{g}", name=f"xg_{g}")
            nc.sync.dma_start(out=xt_, in_=x[g, :, :])
            xg.append(xt_)

        # Dummy activation so walrus places the sigmoid table load here, where
        # it overlaps the input DMAs instead of stalling the first real sigmoid
        warm = consts.tile([1, 1], BF16, tag="warm", name="warm")
        nc.scalar.activation(warm, bs[0:1, 0:1], AF.Sigmoid)

        # h state lives in [HID, 2*BF] pair tiles: subtiles (2j, 2j+1) share one
        h1p = {}
        h2p = {}
        for j in range(NSUB // 2):
            h1p[j] = hpool.tile([HID, 2 * BF], BF16, tag=f"hL1_{j}",
                                name=f"h1i_{j}")
            nc.vector.memset(h1p[j], 0.0)
            h2p[j] = hpool.tile([HID, 2 * BF], BF16, tag=f"hL2_{j}",
                                name=f"h2i_{j}")
            nc.vector.memset(h2p[j], 0.0)

        def h_slice(layer, s):
            hp_ = (h1p if layer == "L1" else h2p)[s // 2]
            return hp_[:, (s % 2) * BF:(s % 2 + 1) * BF]

        # Same-layer pairs of L1(t) and L2(t-1) interleaved: every
        # producer->consumer distance becomes ~16-18 slots (vs 8 with
        # block order), giving the elementwise tail ~10 slots of margin
        # before the consuming matmuls instead of 1.
        cells = []
        for t in range(T + 1):
            for g in range(NSUB // 2):
                if t < T:
                    cells.append(("L1", t, 2 * g))
                    cells.append(("L1", t, 2 * g + 1))
                if t > 0:
                    cells.append(("L2", t - 1, 2 * g))
                    cells.append(("L2", t - 1, 2 * g + 1))
        NCELLS = len(cells)

        def _cell_aps(k, st):
            layer, t, s = cells[k]
            rz = prz.tile([HID, 2, BF], F32, tag="rz", name=f"rz{k}")
            if layer == "L1":
                g, j = divmod(t, 3)
                xa = xg[g][32 * j:32 * j + KAUG, s * BF:(s + 1) * BF]
                kin, wb = KAUG, 32 * j
                wi, wh = w1, wh1
            else:
                xa = h_slice("L1", s)
                kin, wb = HID, 0
                wi, wh = wi2, wh2
            hprev = h_slice(layer, s)
            st.update(rz=rz, layer=layer, t=t, s=s)
            if s % 2 == 0:
                st["hprevp"] = (h1p if layer == "L1" else h2p)[s // 2]
            return xa, kin, wb, wi, wh, hprev

        def mm_stage(k, st, st0):
            xa, kin, wb, wi, wh, hprev = _cell_aps(k, st)
            nc.tensor.matmul(st["rz"][:, 0, :], wi[wb:wb + kin, 0:HID], xa,
                             start=True, stop=False)
            nc.tensor.matmul(st["rz"][:, 0, :], wh[:, 0:HID], hprev,
                             start=False, stop=True)
            nc.tensor.matmul(st["rz"][:, 1, :], wi[wb:wb + kin, HID:2 * HID],
                             xa, start=True, stop=False)
            nc.tensor.matmul(st["rz"][:, 1, :], wh[:, HID:2 * HID], hprev,
                             start=False, stop=True)
            gi = pgi.tile([HID, BF], F32, tag="gi", name=f"gi{k}")
            gh = pgh.tile([HID, BF], F32, tag="gh", name=f"gh{k}")
            nc.tensor.matmul(gi, wi[wb:wb + kin, 2 * HID:3 * HID], xa,
                             start=True, stop=True)
            nc.tensor.matmul(gh, wh[:, 2 * HID:3 * HID], hprev,
                             start=True, stop=True)
            st["gi"], st["gh"] = gi, gh

        def sig_stage(k, st, st0):
            # rzsp pair layout: [HID, 2, 2*BF] = [r(a)|r(b) | z(a)|z(b)]
            if k % 2 == 0:
                st0["rzsp"] = spool.tile([HID, 2, 2 * BF], BF16, tag="rzsp",
                                         bufs=3, name=f"rzsp{k}")
            h = k % 2
            rzsp = st0["rzsp"]
            if st["layer"] == "L1":
                nc.scalar.activation(rzsp[:, :, h * BF:(h + 1) * BF], st["rz"],
                                     AF.Sigmoid)
            else:
                nc.scalar.activation(rzsp[:, 0, h * BF:(h + 1) * BF],
                                     st["rz"][:, 0, :], AF.Sigmoid,
                                     bias=bs[:, 0:1])
                nc.scalar.activation(rzsp[:, 1, h * BF:(h + 1) * BF],
                                     st["rz"][:, 1, :], AF.Sigmoid,
                                     bias=bs[:, 1:2])

        def t1_stage(k, st, st0):
            bhn = bs[:, 2:3] if st["layer"] == "L1" else bs[:, 3:4]
            h = k % 2
            t1 = spool.tile([HID, BF], BF16, tag="t1", bufs=4, name=f"t1_{k}")
            nc.vector.scalar_tensor_tensor(
                t1, st["gh"], bhn,
                st0["rzsp"][:, 0, h * BF:(h + 1) * BF],
                op0=ALU.add, op1=ALU.mult)
            st["t1"] = t1

        def t2_stage(k, st, st0):
            h = k % 2
            if h == 0:
                st0["t2p"] = spool.tile([HID, 2 * BF], BF16, tag="t2p", bufs=3,
                                        name=f"t2p_{k}")
            nc.vector.tensor_tensor(st0["t2p"][:, h * BF:(h + 1) * BF],
                                    st["t1"], st["gi"], op=ALU.add)

        def tanh_pair(k0, st0):
            nsbp = spool.tile([HID, 2 * BF], BF16, tag="nsbp", bufs=3,
                              name=f"nsbp{k0}")
            tb = 0.0 if st0["layer"] == "L1" else bs[:, 4:5]
            nc.scalar.activation(nsbp, st0["t2p"], AF.Tanh, bias=tb)
            st0["nsbp"] = nsbp

        def t3p_stage(k0, st0):
            t3p = spool.tile([HID, 2 * BF], BF16, tag="t3p", bufs=3,
                             name=f"t3p_{k0}")
            nc.vector.tensor_tensor(t3p, st0["hprevp"], st0["nsbp"],
                                    op=ALU.subtract)
            st0["t3p"] = t3p

        def t4p_stage(k0, st0):
            t4p = spool.tile([HID, 2 * BF], BF16, tag="t4p", bufs=3,
                             name=f"t4p_{k0}")
            nc.vector.tensor_tensor(t4p, st0["rzsp"][:, 1, :], st0["t3p"],
                                    op=ALU.mult)
            st0["t4p"] = t4p

        def hpp_stage(k0, st0):
            hprep = spool.tile([HID, 2 * BF], BF16, tag="hprep", bufs=3,
                               name=f"hprep{k0}")
            nc.vector.tensor_tensor(hprep, st0["t4p"], st0["nsbp"], op=ALU.add)
            st0["hprep"] = hprep

        def relu_pair(k0, st0):
            layer, s = st0["layer"], st0["s"]
            j = s // 2
            hnp = hpool.tile([HID, 2 * BF], BF16, tag=f"h{layer}_{j}",
                             name=f"h{layer}_{j}_{k0}")
            nc.scalar.activation(hnp, st0["hprep"], AF.Relu)
            if layer == "L1":
                h1p[j] = hnp
            else:
                h2p[j] = hnp

        states = {}
        for k in range(NCELLS + 7):
            if k < NCELLS:
                st = states[k] = {}
                mm_stage(k, st, states[k // 2 * 2])
            if k - 4 >= 0 and (k - 4) % 2 == 0 and k - 4 < NCELLS:
                tanh_pair(k - 4, states[k - 4])
            if k < NCELLS:
                sig_stage(k, states[k], states[k // 2 * 2])
            if k - 7 >= 0 and (k - 7) % 2 == 0 and k - 7 < NCELLS:
                relu_pair(k - 7, states[k - 7])
                del states[k - 7], states[k - 6]
            if k - 5 >= 0 and (k - 5) % 2 == 0 and k - 5 < NCELLS:
                t3p_stage(k - 5, states[k - 5])
                t4p_stage(k - 5, states[k - 5])
                hpp_stage(k - 5, states[k - 5])
            if 0 <= k - 1 < NCELLS:
                t1_stage(k - 1, states[k - 1], states[(k - 1) // 2 * 2])
                t2_stage(k - 1, states[k - 1], states[(k - 1) // 2 * 2])

        ob = opool.tile([NOUT, B_CORE], F32, tag="ob")
        for s in range(NSUB):
            po = pgi.tile([NOUT, BF], F32, tag="gi", name=f"po{s}")
            nc.tensor.matmul(po, wo, h_slice("L2", s), start=True, stop=True)
            nc.vector.tensor_tensor(ob[:, s * BF:(s + 1) * BF], po, bo[:, 0:BF],
                                    op=ALU.add)
        nc.scalar.dma_start(out=out_d[:, :], in_=ob)

    return nc


def _prep_inputs(x, w_ih1, w_hh1, b_ih1, b_hh1, w_ih2, w_hh2, b_ih2, b_hh2,
                 w_out, b_out):
    """Host-side reshape/transpose/cast + per-core sharding."""
    n = N_TOTAL
    xs = np.asarray(x, np.float32).reshape(n, T, C)       # channel dim is 1
    xt = np.transpose(xs, (1, 2, 0))                      # [T, C, n]
    xg = np.zeros(((T + 2) // 3, 128, n), np.float32)
    for t in range(T):
        g, j = divmod(t, 3)
        xg[g, 32 * j:32 * j + C, :] = xt[t]
        xg[g, 32 * j + C, :] = 1.0
    xg16 = xg.astype(ml_dtypes.bfloat16)

    w_ih1 = np.asarray(w_ih1, np.float32)
    w_hh1 = np.asarray(w_hh1, np.float32)
    b_ih1 = np.asarray(b_ih1, np.float32)
    b_hh1 = np.asarray(b_hh1, np.float32)
    w_ih2 = np.asarray(w_ih2, np.float32)
    w_hh2 = np.asarray(w_hh2, np.float32)
    b_ih2 = np.asarray(b_ih2, np.float32)
    b_hh2 = np.asarray(b_hh2, np.float32)
    w_out = np.asarray(w_out, np.float32)
    b_out = np.asarray(b_out, np.float32)

    H = HID
    w1aug = np.zeros((128, 3 * H), np.float32)
    bias_row = np.concatenate([
        b_ih1[0:H] + b_hh1[0:H],          # r: both biases
        b_ih1[H:2 * H] + b_hh1[H:2 * H],  # z: both biases
        b_ih1[2 * H:3 * H],               # n: input-side bias only
    ])
    for j in range(4):
        w1aug[32 * j:32 * j + C, :] = w_ih1.T
        w1aug[32 * j + C, :] = bias_row

    biases = np.stack([
        b_ih2[0:H] + b_hh2[0:H],
        b_ih2[H:2 * H] + b_hh2[H:2 * H],
        b_hh1[2 * H:3 * H],
        b_hh2[2 * H:3 * H],
        b_ih2[2 * H:3 * H],
    ], axis=1).astype(np.float32)         # [H, 5]

    common = {
        "w1aug": np.ascontiguousarray(w1aug.astype(ml_dtypes.bfloat16)),
        "whh1T": np.ascontiguousarray(w_hh1.T.astype(ml_dtypes.bfloat16)),
        "wih2T": np.ascontiguousarray(w_ih2.T.astype(ml_dtypes.bfloat16)),
        "whh2T": np.ascontiguousarray(w_hh2.T.astype(ml_dtypes.bfloat16)),
        "woutT": np.ascontiguousarray(w_out.T.astype(ml_dtypes.bfloat16)),
        "biases": np.ascontiguousarray(biases),
        "bout": np.ascontiguousarray(
            np.broadcast_to(b_out.reshape(NOUT, 1), (NOUT, 2 * BF)).astype(np.float32)),
    }
    in_maps = []
    for c in range(NCORES):
        m = dict(common)
        m["x"] = np.ascontiguousarray(xg16[:, :, c * B_CORE:(c + 1) * B_CORE])
        in_maps.append(m)
    return in_maps


def kernel(**inputs):
    global LAST_RESULT
    nc = _build_bass()
    edited = _split_multi_waits(nc.to_json_bytes())
    nc.to_json_bytes = lambda: edited
    in_maps = _prep_inputs(**inputs)
    trace = bool(int(os.environ.get("BASS_TRACE", "0")))
    res = run_bass_kernel_spmd(nc, in_maps, core_ids=list(range(NCORES)),
                               trace=trace)
    LAST_RESULT = res
    outs = [r["out"] for r in res.results]          # each [NOUT, B_CORE] f32
    full = np.concatenate(outs, axis=1)             # [NOUT, N_TOTAL]
    return np.ascontiguousarray(full.T).astype(np.float32)

